# revision 9
# baseline (speedup 1.0000x reference)
"""Trainium2 Bass kernel for nn_Attention_2 (gnn_message_passing).

Pure data parallel over the batch/node dim B=32768: 8 NeuronCores each
process 4096 rows. Per 256-row super-tile, the per-head softmax/gate
pipeline runs in a transposed layout ((h,j) on partitions, b on free dim)
so every reduction is a TensorEngine matmul against tiny host-built
constants; the memory-dominant aggregation over neighbors is 32 small
matmuls per 128-row half with the per-row weights as a 4-live-column
stationary operand and host-pre-cast bf16 context as the moving operand,
accumulating straight into PSUM.

v3: software-pipelined — the softmax/gate chain of super-tile t is
interleaved into the aggregation matmul stream of super-tile t-1 so the
TensorEngine never stalls on the ACT/DVE round trips between the chain's
small matmuls. Context is pre-cast to bf16 on the host (half-width DRAM
read), output is written bf16 and upcast on the host. All small constants
ride in one packed [128,646] DMA, the whole source_distance input is
preloaded in one DMA, and DMA triggers are spread over sync (context),
scalar (consts/sd) and gpsimd (output) so no engine's descriptor
generation stalls another's.
"""

import sys

for _p in ("/opt/trn_rl_repo", "/root/.axon_site/_ro/trn_rl_repo"):
    if _p not in sys.path:
        sys.path.insert(0, _p)

from contextlib import ExitStack

import numpy as np

import concourse.bass as bass
import concourse.mybir as mybir
import concourse.tile as tile
from concourse import bacc
from concourse.bass_utils import run_bass_kernel_spmd

# Problem shape (hardcoded; kernel.py must be self-contained)
B, K, D, H = 32768, 32, 192, 4
NCORES = 8
ROWS = B // NCORES          # 4096 rows per core
P = 128                     # partitions / rows per tile
NT = ROWS // P              # 32 tiles per core
G = 4                       # rows per aggregation block (G*K == P)
NB = P // G                 # 32 blocks per tile
HK = H * K                  # 128
ST = 2                      # tiles per super-tile (256-row softmax/gate chain)

F32 = mybir.dt.float32
BF16 = mybir.dt.bfloat16
REGW = NB * (P + G)  # 4224: aggregation stationary-weight region width

# packed-constant column offsets in c_all [128, CW]
C_KERN, C_BIAS, C_BLK, C_E4, C_GD, C_GBH, C_HG, C_MASK = (
    0, 128, 129, 133, 261, 389, 390, 518)
CW = 646

_CACHE: dict = {}


def build_program(nt: int = NT):
    rows = nt * P
    nc = bacc.Bacc("TRN2", target_bir_lowering=False, debug=False, num_devices=NCORES)

    # Host-pretransposed inputs: sd as [K, rows] and ctx (bf16) as
    # [P, nt*NB*D] with ctx_host[p, (t, j, d)] = context[b0(t) + 4j + p//K,
    # p%K, d] — so every per-tile DMA reads one contiguous run per partition.
    F32R = mybir.dt.float32r
    sd_d = nc.dram_tensor("sd", [K, rows], F32, kind="ExternalInput").ap()
    ctx_d = nc.dram_tensor("ctx", [P, nt * NB * D], BF16, kind="ExternalInput").ap()
    call_d = nc.dram_tensor("c_all", [P, CW], mybir.dt.float32r, kind="ExternalInput").ap()
    out_d = nc.dram_tensor("out", [rows, D], BF16, kind="ExternalOutput").ap()

    with tile.TileContext(nc) as tc, ExitStack() as ctx:
        consts = ctx.enter_context(tc.tile_pool(name="consts", bufs=1))
        ctbp = ctx.enter_context(tc.tile_pool(name="ctbp", bufs=4))
        smallp = ctx.enter_context(tc.tile_pool(name="smallp", bufs=10))
        outp = ctx.enter_context(tc.tile_pool(name="outp", bufs=4))
        ps_mm = ctx.enter_context(tc.tile_pool(name="ps_mm", bufs=5, space="PSUM"))
        ps_out = ctx.enter_context(tc.tile_pool(name="ps_out", bufs=3, space="PSUM"))

        # Stationary-weight regions for the aggregation matmuls: 32 windows of
        # 128 bf16 columns spaced 132 apart; window j's only nonzero columns
        # are 4j..4j+3 (at col offset 136j), rewritten every tile. The rest
        # stays zero from the one-time memsets (f32-bitcast for 2x rate, split
        # across vector+gpsimd so they overlap the leading DMAs).
        regions = []
        for ri in range(4):
            reg = consts.tile([P, REGW], BF16, name=f"agg_region{ri}")
            regions.append(reg)
            eng = nc.vector if ri % 2 == 0 else nc.gpsimd
            eng.memset(reg[:].bitcast(F32), 0.0)

        c_all = consts.tile([P, CW], F32R)
        nc.scalar.dma_start(c_all[:], call_d)
        sd_all = consts.tile([K, rows], F32)
        nc.scalar.dma_start(sd_all[:], sd_d)

        c_kern = c_all[0:K, C_KERN:C_KERN + HK]
        c_bias = c_all[:, C_BIAS:C_BIAS + 1].bitcast(F32)
        c_blk = c_all[:, C_BLK:C_BLK + H]
        c_e4 = c_all[0:H, C_E4:C_E4 + HK].bitcast(F32)
        c_gd = c_all[:, C_GD:C_GD + HK]
        c_gbh = c_all[:, C_GBH:C_GBH + 1].bitcast(F32)
        c_hg = c_all[:, C_HG:C_HG + P]
        c_mask = c_all[:, C_MASK:C_MASK + P].bitcast(F32)

        def region_write_view(reg):
            # [128, 32, 4] view hitting cols 136j + i (the live columns of
            # window j, which starts at col 132j)
            return reg[:].rearrange("p (j x) -> p j x", x=G)[:, 0:REGW // G:(P + 2 * G) // G, :]

        mview = c_mask.rearrange("p (j x) -> p j x", x=G)

        def chain_front(t):
            # sq + simi for super-tile t (DVE+ACT only; sd preloaded)
            SP = ST * P
            sq = smallp.tile([K, SP], F32, tag="sm")
            nc.vector.tensor_mul(sq[:], sd_all[:, t * P:t * P + SP],
                                 sd_all[:, t * P:t * P + SP])
            simi_T = smallp.tile([K, SP], F32R, tag="sm")
            nc.scalar.activation(simi_T[:], sq[:],
                                 mybir.ActivationFunctionType.Exp, scale=-0.5)
            return simi_T

        def agg_gen(hregs, ctb):
            # Deferred aggregation matmuls for one super-tile: per 128-row
            # half, 32 PSUM-accumulating matmuls (window j's stationary weight
            # has nonzeros only in out-partition columns 4j..4j+3).
            out_pss = []
            for hh in range(ST):
                reg = hregs[hh]
                out_ps = ps_out.tile([P, D], F32, tag="outps")
                out_pss.append(out_ps)
                for j in range(NB):
                    def mm(j=j, hh=hh, reg=reg, out_ps=out_ps):
                        nc.tensor.matmul(
                            out_ps[:],
                            lhsT=reg[:, (P + G) * j:(P + G) * j + P],
                            rhs=ctb[:, (hh * NB + j) * D:(hh * NB + j + 1) * D],
                            start=(j == 0), stop=(j == NB - 1))
                    yield mm
            yield out_pss

        def drain(g, n):
            got = None
            for _ in range(n):
                op = next(g, None)
                if op is None:
                    return got
                if callable(op):
                    op()
                else:
                    got = op
            return got

        def emit_fins(out_pss, r0):
            # PSUM->SBUF copies (bf16 cast) + output DMAs, issued after the
            # chain's DVE ops so they never delay the reciprocal path; DMA
            # triggered from gpsimd so sync/scalar descriptor-gen is never
            # blocked waiting on the copies.
            for hh, out_ps in enumerate(out_pss):
                out_sb = outp.tile([P, D], BF16)
                nc.vector.tensor_copy(out_sb[:], out_ps[:])
                nc.gpsimd.dma_start(out_d[r0 + hh * P:r0 + (hh + 1) * P, :],
                                    out_sb[:])

        pending = iter(())
        simi_cur = None

        assert nt % ST == 0
        for t in range(0, nt, ST):
            r0 = t * P
            SP = ST * P

            # chunk-major bf16 context super-tile, one DMA (the only sync
            # engine work, so context descriptors always issue back-to-back)
            ctb = ctbp.tile([P, ST * NB * D], BF16)
            nc.sync.dma_start(ctb[:], ctx_d[:, t * NB * D:(t + ST) * NB * D])

            if simi_cur is None:
                simi_cur = chain_front(t)

            drain(pending, 6)
            # logits_T[(h,j), b] then p = exp(logits + bias)
            logits_ps = ps_mm.tile([HK, SP], F32, tag="mm")
            nc.tensor.matmul(logits_ps[:], lhsT=c_kern, rhs=simi_cur[:])
            drain(pending, 13)
            p_t = smallp.tile([HK, SP], F32R, tag="sm")
            nc.scalar.activation(p_t[:], logits_ps[:],
                                 mybir.ActivationFunctionType.Exp, bias=c_bias)
            p_tf = p_t[:].bitcast(F32)

            # per-(h,b) softmax denominator and its reciprocal, broadcast back
            s_ps = ps_mm.tile([H, SP], F32, tag="mm")
            nc.tensor.matmul(s_ps[:], lhsT=c_blk, rhs=p_t[:])
            drain(pending, 13)
            rs = smallp.tile([H, SP], F32, tag="sm")
            nc.vector.reciprocal_approx_fast(out=rs[:], in_=s_ps[:])
            sbc_ps = ps_mm.tile([HK, SP], F32, tag="mm")
            nc.tensor.matmul(sbc_ps[:], lhsT=c_e4, rhs=rs[:])
            drain(pending, 13)
            w_t = smallp.tile([HK, SP], F32R, tag="sm")
            nc.vector.tensor_mul(w_t[:], p_tf, sbc_ps[:])

            # gate: sigmoid(x) = 0.5*(1+tanh(x/2)); the 0.5 is folded into hg4h
            gl_ps = ps_mm.tile([HK, SP], F32, tag="mm")
            nc.tensor.matmul(gl_ps[:], lhsT=c_gd, rhs=w_t[:])
            drain(pending, 13)
            th = smallp.tile([HK, SP], F32, tag="sm")
            nc.scalar.activation(th[:], gl_ps[:],
                                 mybir.ActivationFunctionType.Tanh,
                                 bias=c_gbh, scale=0.5)
            gated2 = smallp.tile([HK, SP], F32R, tag="sm")
            nc.vector.scalar_tensor_tensor(
                out=gated2[:], in0=th[:], scalar=1.0, in1=w_t[:].bitcast(F32),
                op0=mybir.AluOpType.add, op1=mybir.AluOpType.mult)

            # issue next super-tile's sq/simi now so its MM1 never waits on
            # this super-tile's trailing DVE work
            simi_next = chain_front(t + ST) if t + ST < nt else None

            # head-combine (replicated 4x over row-groups), then block-mask the
            # live columns straight into each half-tile's stationary region
            wrep_ps = ps_mm.tile([P, SP], F32, tag="mm")
            nc.tensor.matmul(wrep_ps[:], lhsT=c_hg, rhs=gated2[:])
            out_pss_prev = drain(pending, 1 << 30)
            hregs = []
            for hh in range(ST):
                reg = regions[(t + hh) % 4]
                wview = wrep_ps[:, hh * P:(hh + 1) * P].rearrange(
                    "p (j x) -> p j x", x=G)
                nc.vector.tensor_mul(region_write_view(reg), wview, mview)
                hregs.append(reg)

            if out_pss_prev:
                emit_fins(out_pss_prev, r0 - ST * P)
            pending = agg_gen(hregs, ctb)
            simi_cur = simi_next

        out_pss_last = drain(pending, 1 << 30)
        if out_pss_last:
            emit_fins(out_pss_last, (nt - ST) * P)

    nc.compile()
    return nc


def _softmax(x):
    e = np.exp(x - x.max())
    return e / e.sum()


def build_consts(kernels, biases, gate_W, gate_b, gate_weights, gate_bias):
    f32 = np.float32
    kern_r = np.ascontiguousarray(kernels.transpose(1, 0, 2).reshape(K, HK)).astype(f32)
    hg = _softmax(np.asarray(gate_weights, np.float64) + np.asarray(gate_bias, np.float64))
    c_all = np.zeros((P, CW), f32)
    c_all[0:K, C_KERN:C_KERN + HK] = kern_r
    c_all[:, C_BIAS] = biases.reshape(HK)
    c_all[:, C_BLK:C_BLK + H] = np.kron(np.eye(H), np.ones((K, 1)))
    c_all[0:H, C_E4:C_E4 + HK] = np.kron(np.eye(H), np.ones((1, K)))
    c_all[:, C_GD:C_GD + HK] = np.kron(np.eye(H), gate_W)
    c_all[:, C_GBH] = 0.5 * np.tile(gate_b, H)
    c_all[:, C_HG:C_HG + P] = np.kron((0.5 * hg)[:, None] @ np.ones((1, H)), np.eye(K))
    c_all[:, C_MASK:C_MASK + P] = (
        np.arange(P)[:, None] // K == np.arange(P)[None, :] % G)
    return c_all


def run(inputs: dict, trace: bool = False, **kw):
    """inputs: full-size arrays keyed as in setup_inputs(). Returns (out, results)."""
    import ml_dtypes

    if "nc" not in _CACHE:
        _CACHE["nc"] = build_program()
    nc = _CACHE["nc"]

    sd = np.ascontiguousarray(np.asarray(inputs["source_distance"], np.float32))
    ctx = np.asarray(inputs["context"], np.float32).astype(ml_dtypes.bfloat16)
    c_all = build_consts(
        np.asarray(inputs["kernels"], np.float32),
        np.asarray(inputs["biases"], np.float32),
        np.asarray(inputs["gate_W"], np.float32),
        np.asarray(inputs["gate_b"], np.float32),
        np.asarray(inputs["gate_weights"], np.float32),
        np.asarray(inputs["gate_bias"], np.float32),
    )

    in_maps = []
    for c in range(NCORES):
        b0 = c * ROWS
        # host-side layout transforms so every device DMA run is long+contiguous
        sd_c = np.ascontiguousarray(sd[b0:b0 + ROWS].T)                    # [K, ROWS]
        ctx_c = np.ascontiguousarray(
            ctx[b0:b0 + ROWS].reshape(NT, NB, P, D).transpose(2, 0, 1, 3)
        ).reshape(P, NT * NB * D)
        in_maps.append({"sd": sd_c, "ctx": ctx_c, "c_all": c_all})

    results = run_bass_kernel_spmd(nc, in_maps, core_ids=list(range(NCORES)),
                                   trace=trace, **kw)
    out = np.concatenate(
        [results.results[c]["out"].astype(np.float32) for c in range(NCORES)],
        axis=0)
    return out, results


def kernel(**inputs) -> np.ndarray:
    out, _ = run(inputs)
    return out


# revision 11
# speedup vs baseline: 1.1838x; 1.1838x over previous
"""Trainium2 Bass kernel for nn_Attention_2 (gnn_message_passing).

Pure data parallel over the batch/node dim B=32768: 8 NeuronCores each
process 4096 rows. Per 256-row super-tile, the per-head softmax/gate
pipeline runs in a transposed layout ((h,j) on partitions, b on free dim)
so every reduction is a TensorEngine matmul against tiny host-built
constants; the memory-dominant aggregation over neighbors is 32 small
matmuls per 128-row half with the per-row weights as a 4-live-column
stationary operand and host-pre-cast bf16 context as the moving operand,
accumulating straight into PSUM.

v4: two-stage software pipeline. The softmax/gate chain has ~5 serial
PE<->ACT/DVE round trips at ~2us apiece (semaphore propagation dominates),
which exceeds one super-tile's aggregation work (~7us) — so each chain is
spread across TWO loop iterations (MM1/exp/MM2 for super-tile t; recip
through MM5 for t-2) and the aggregation matmuls of t-4 are drained into
every gap of the PE instruction stream. Context is bf16 (host pre-cast),
output is written bf16 and upcast on the host, all constants ride in one
packed DMA, source_distance is preloaded whole, and DMA triggers are
split: sync=inputs, gpsimd=outputs, so descriptor generation for the
context stream is never blocked.
"""

import sys

for _p in ("/opt/trn_rl_repo", "/root/.axon_site/_ro/trn_rl_repo"):
    if _p not in sys.path:
        sys.path.insert(0, _p)

from contextlib import ExitStack

import numpy as np

import concourse.bass as bass
import concourse.mybir as mybir
import concourse.tile as tile
from concourse import bacc
from concourse.bass_utils import run_bass_kernel_spmd

# Problem shape (hardcoded; kernel.py must be self-contained)
B, K, D, H = 32768, 32, 192, 4
NCORES = 8
ROWS = B // NCORES          # 4096 rows per core
P = 128                     # partitions / rows per tile
NT = ROWS // P              # 32 tiles per core
G = 4                       # rows per aggregation block (G*K == P)
NB = P // G                 # 32 blocks per tile
HK = H * K                  # 128
ST = 2                      # tiles per super-tile (256-row softmax/gate chain)
SP = ST * P

F32 = mybir.dt.float32
BF16 = mybir.dt.bfloat16
REGW = NB * (P + G)  # 4224: aggregation stationary-weight region width

# packed-constant column offsets in c_all [128, CW]
C_KERN, C_BIAS, C_BLK, C_E4, C_GD, C_GBH, C_HG, C_MASK = (
    0, 128, 129, 133, 261, 389, 390, 518)
CW = 646

_CACHE: dict = {}


def build_program(nt: int = NT):
    rows = nt * P
    nc = bacc.Bacc("TRN2", target_bir_lowering=False, debug=False, num_devices=NCORES)

    # Host-pretransposed inputs: sd as [K, rows] and ctx (bf16) as
    # [P, nt*NB*D] with ctx_host[p, (t, j, d)] = context[b0(t) + 4j + p//K,
    # p%K, d] — so every per-tile DMA reads one contiguous run per partition.
    F32R = mybir.dt.float32r
    sd_d = nc.dram_tensor("sd", [K, rows], F32, kind="ExternalInput").ap()
    ctx_d = nc.dram_tensor("ctx", [P, nt * NB * D], BF16, kind="ExternalInput").ap()
    call_d = nc.dram_tensor("c_all", [P, CW], F32R, kind="ExternalInput").ap()
    out_d = nc.dram_tensor("out", [rows, D], BF16, kind="ExternalOutput").ap()

    with tile.TileContext(nc) as tc, ExitStack() as ctx:
        consts = ctx.enter_context(tc.tile_pool(name="consts", bufs=1))
        ctbp = ctx.enter_context(tc.tile_pool(name="ctbp", bufs=4))
        smallp = ctx.enter_context(tc.tile_pool(name="smallp", bufs=12))
        outp = ctx.enter_context(tc.tile_pool(name="outp", bufs=4))
        ps_mm = ctx.enter_context(tc.tile_pool(name="ps_mm", bufs=4, space="PSUM"))
        ps_s = ctx.enter_context(tc.tile_pool(name="ps_s", bufs=2, space="PSUM"))
        ps_out = ctx.enter_context(tc.tile_pool(name="ps_out", bufs=2, space="PSUM"))

        # Stationary-weight regions for the aggregation matmuls: 32 windows of
        # 128 bf16 columns spaced 132 apart; window j's only nonzero columns
        # are 4j..4j+3 (at col offset 136j), rewritten every tile. The rest
        # stays zero from the one-time memsets (f32-bitcast for 2x rate, split
        # across vector+gpsimd so they overlap the leading DMAs).
        regions = []
        for ri in range(4):
            reg = consts.tile([P, REGW], BF16, name=f"agg_region{ri}")
            regions.append(reg)
            eng = nc.vector if ri % 2 == 0 else nc.gpsimd
            eng.memset(reg[:].bitcast(F32), 0.0)

        # input DMAs all on sync, consts/sd first so their descriptors are
        # processed before the deep context prefetch queue
        c_all = consts.tile([P, CW], F32R)
        nc.sync.dma_start(c_all[:], call_d)
        sd_all = consts.tile([K, rows], F32)
        nc.sync.dma_start(sd_all[:], sd_d)

        c_kern = c_all[0:K, C_KERN:C_KERN + HK]
        c_bias = c_all[:, C_BIAS:C_BIAS + 1].bitcast(F32)
        c_blk = c_all[:, C_BLK:C_BLK + H]
        c_e4 = c_all[0:H, C_E4:C_E4 + HK].bitcast(F32)
        c_gd = c_all[:, C_GD:C_GD + HK]
        c_gbh = c_all[:, C_GBH:C_GBH + 1].bitcast(F32)
        c_hg = c_all[:, C_HG:C_HG + P]
        c_mask = c_all[:, C_MASK:C_MASK + P].bitcast(F32)

        def region_write_view(reg):
            # [128, 32, 4] view hitting cols 136j + i (the live columns of
            # window j, which starts at col 132j)
            return reg[:].rearrange("p (j x) -> p j x", x=G)[:, 0:REGW // G:(P + 2 * G) // G, :]

        mview = c_mask.rearrange("p (j x) -> p j x", x=G)

        def chain_front(t):
            # sq + simi for super-tile t (DVE+ACT only; sd preloaded)
            sq = smallp.tile([K, SP], F32, tag="sm")
            nc.vector.tensor_mul(sq[:], sd_all[:, t * P:t * P + SP],
                                 sd_all[:, t * P:t * P + SP])
            simi_T = smallp.tile([K, SP], F32R, tag="sm")
            nc.scalar.activation(simi_T[:], sq[:],
                                 mybir.ActivationFunctionType.Exp, scale=-0.5)
            return simi_T

        def agg_gen(hregs, ctb, r0):
            # Deferred aggregation matmuls for one super-tile: per 128-row
            # half, 32 PSUM-accumulating matmuls (window j's stationary weight
            # has nonzeros only in out-partition columns 4j..4j+3).
            out_pss = []
            for hh in range(ST):
                reg = hregs[hh]
                out_ps = ps_out.tile([P, D], F32, tag="outps")
                out_pss.append(out_ps)
                for j in range(NB):
                    def mm(j=j, hh=hh, reg=reg, out_ps=out_ps):
                        nc.tensor.matmul(
                            out_ps[:],
                            lhsT=reg[:, (P + G) * j:(P + G) * j + P],
                            rhs=ctb[:, (hh * NB + j) * D:(hh * NB + j + 1) * D],
                            start=(j == 0), stop=(j == NB - 1))
                    yield mm
            yield (out_pss, r0)

        def drain(g, n):
            got = None
            for _ in range(n):
                op = next(g, None)
                if op is None:
                    return got
                if callable(op):
                    op()
                else:
                    got = op
            return got

        def emit_fins(fin):
            # PSUM->SBUF copies (bf16 cast) + output DMAs, issued after the
            # chain's DVE ops so they never delay the reciprocal path; DMA
            # triggered from gpsimd so input descriptor-gen is never blocked
            # waiting on the copies.
            out_pss, r0 = fin
            for hh, out_ps in enumerate(out_pss):
                out_sb = outp.tile([P, D], BF16)
                nc.vector.tensor_copy(out_sb[:], out_ps[:])
                nc.gpsimd.dma_start(out_d[r0 + hh * P:r0 + (hh + 1) * P, :],
                                    out_sb[:])

        # pipeline state
        pending = iter(())      # aggregation stream of super-tile t-4
        simi_cur = None         # simi(t), made last iteration
        back = None             # (prevt, s_ps, p_t, ctb) : chain back half input

        assert nt % ST == 0
        for t in range(0, nt, ST):
            # chunk-major bf16 context super-tile, one DMA
            ctb = ctbp.tile([P, ST * NB * D], BF16)
            nc.sync.dma_start(ctb[:], ctx_d[:, t * NB * D:(t + ST) * NB * D])

            if simi_cur is None:
                simi_cur = chain_front(t)

            # ---- chain stage A for super-tile t: MM1 -> exp -> MM2
            drain(pending, 6)
            logits_ps = ps_mm.tile([HK, SP], F32, tag="mm")
            nc.tensor.matmul(logits_ps[:], lhsT=c_kern, rhs=simi_cur[:])
            p_t = smallp.tile([HK, SP], F32R, tag="sm")
            nc.scalar.activation(p_t[:], logits_ps[:],
                                 mybir.ActivationFunctionType.Exp, bias=c_bias)
            drain(pending, 14)
            s_ps = ps_s.tile([H, SP], F32, tag="s")
            nc.tensor.matmul(s_ps[:], lhsT=c_blk, rhs=p_t[:])
            drain(pending, 8)

            # ---- chain stage B for super-tile t-2: recip -> ... -> MM5
            fin = None
            if back is not None:
                prevt, s_prev, p_prev, ctb_prev = back
                rs = smallp.tile([H, SP], F32, tag="sm")
                nc.vector.reciprocal_approx_fast(out=rs[:], in_=s_prev[:])
                sbc_ps = ps_mm.tile([HK, SP], F32, tag="mm")
                nc.tensor.matmul(sbc_ps[:], lhsT=c_e4, rhs=rs[:])
                drain(pending, 8)
                w_t = smallp.tile([HK, SP], F32R, tag="sm")
                nc.vector.tensor_mul(w_t[:], p_prev[:].bitcast(F32), sbc_ps[:])
                gl_ps = ps_mm.tile([HK, SP], F32, tag="mm")
                nc.tensor.matmul(gl_ps[:], lhsT=c_gd, rhs=w_t[:])
                drain(pending, 8)
                th = smallp.tile([HK, SP], F32, tag="sm")
                nc.scalar.activation(th[:], gl_ps[:],
                                     mybir.ActivationFunctionType.Tanh,
                                     bias=c_gbh, scale=0.5)
                gated2 = smallp.tile([HK, SP], F32R, tag="sm")
                nc.vector.scalar_tensor_tensor(
                    out=gated2[:], in0=th[:], scalar=1.0,
                    in1=w_t[:].bitcast(F32),
                    op0=mybir.AluOpType.add, op1=mybir.AluOpType.mult)
                simi_next = chain_front(t + ST) if t + ST < nt else None
                wrep_ps = ps_mm.tile([P, SP], F32, tag="mm")
                nc.tensor.matmul(wrep_ps[:], lhsT=c_hg, rhs=gated2[:])
                fin = drain(pending, 1 << 30)
                hregs = []
                for hh in range(ST):
                    reg = regions[(prevt + hh) % 4]
                    wview = wrep_ps[:, hh * P:(hh + 1) * P].rearrange(
                        "p (j x) -> p j x", x=G)
                    nc.vector.tensor_mul(region_write_view(reg), wview, mview)
                    hregs.append(reg)
                if fin:
                    emit_fins(fin)
                pending = agg_gen(hregs, ctb_prev, prevt * P)
            else:
                simi_next = chain_front(t + ST) if t + ST < nt else None

            back = (t, s_ps, p_t, ctb)
            simi_cur = simi_next

        # ---- epilogue: chain stage B for the last super-tile, then drain all
        prevt, s_prev, p_prev, ctb_prev = back
        rs = smallp.tile([H, SP], F32, tag="sm")
        nc.vector.reciprocal_approx_fast(out=rs[:], in_=s_prev[:])
        sbc_ps = ps_mm.tile([HK, SP], F32, tag="mm")
        nc.tensor.matmul(sbc_ps[:], lhsT=c_e4, rhs=rs[:])
        drain(pending, 12)
        w_t = smallp.tile([HK, SP], F32R, tag="sm")
        nc.vector.tensor_mul(w_t[:], p_prev[:].bitcast(F32), sbc_ps[:])
        gl_ps = ps_mm.tile([HK, SP], F32, tag="mm")
        nc.tensor.matmul(gl_ps[:], lhsT=c_gd, rhs=w_t[:])
        drain(pending, 12)
        th = smallp.tile([HK, SP], F32, tag="sm")
        nc.scalar.activation(th[:], gl_ps[:],
                             mybir.ActivationFunctionType.Tanh,
                             bias=c_gbh, scale=0.5)
        gated2 = smallp.tile([HK, SP], F32R, tag="sm")
        nc.vector.scalar_tensor_tensor(
            out=gated2[:], in0=th[:], scalar=1.0, in1=w_t[:].bitcast(F32),
            op0=mybir.AluOpType.add, op1=mybir.AluOpType.mult)
        wrep_ps = ps_mm.tile([P, SP], F32, tag="mm")
        nc.tensor.matmul(wrep_ps[:], lhsT=c_hg, rhs=gated2[:])
        fin = drain(pending, 1 << 30)
        hregs = []
        for hh in range(ST):
            reg = regions[(prevt + hh) % 4]
            wview = wrep_ps[:, hh * P:(hh + 1) * P].rearrange(
                "p (j x) -> p j x", x=G)
            nc.vector.tensor_mul(region_write_view(reg), wview, mview)
            hregs.append(reg)
        if fin:
            emit_fins(fin)
        pending = agg_gen(hregs, ctb_prev, prevt * P)
        fin = drain(pending, 1 << 30)
        if fin:
            emit_fins(fin)

    nc.compile()
    return nc


def _softmax(x):
    e = np.exp(x - x.max())
    return e / e.sum()


def build_consts(kernels, biases, gate_W, gate_b, gate_weights, gate_bias):
    f32 = np.float32
    kern_r = np.ascontiguousarray(kernels.transpose(1, 0, 2).reshape(K, HK)).astype(f32)
    hg = _softmax(np.asarray(gate_weights, np.float64) + np.asarray(gate_bias, np.float64))
    c_all = np.zeros((P, CW), f32)
    c_all[0:K, C_KERN:C_KERN + HK] = kern_r
    c_all[:, C_BIAS] = biases.reshape(HK)
    c_all[:, C_BLK:C_BLK + H] = np.kron(np.eye(H), np.ones((K, 1)))
    c_all[0:H, C_E4:C_E4 + HK] = np.kron(np.eye(H), np.ones((1, K)))
    c_all[:, C_GD:C_GD + HK] = np.kron(np.eye(H), gate_W)
    c_all[:, C_GBH] = 0.5 * np.tile(gate_b, H)
    c_all[:, C_HG:C_HG + P] = np.kron((0.5 * hg)[:, None] @ np.ones((1, H)), np.eye(K))
    c_all[:, C_MASK:C_MASK + P] = (
        np.arange(P)[:, None] // K == np.arange(P)[None, :] % G)
    return c_all


def run(inputs: dict, trace: bool = False, **kw):
    """inputs: full-size arrays keyed as in setup_inputs(). Returns (out, results)."""
    import ml_dtypes

    if "nc" not in _CACHE:
        _CACHE["nc"] = build_program()
    nc = _CACHE["nc"]

    sd = np.ascontiguousarray(np.asarray(inputs["source_distance"], np.float32))
    ctx = np.asarray(inputs["context"], np.float32).astype(ml_dtypes.bfloat16)
    c_all = build_consts(
        np.asarray(inputs["kernels"], np.float32),
        np.asarray(inputs["biases"], np.float32),
        np.asarray(inputs["gate_W"], np.float32),
        np.asarray(inputs["gate_b"], np.float32),
        np.asarray(inputs["gate_weights"], np.float32),
        np.asarray(inputs["gate_bias"], np.float32),
    )

    in_maps = []
    for c in range(NCORES):
        b0 = c * ROWS
        # host-side layout transforms so every device DMA run is long+contiguous
        sd_c = np.ascontiguousarray(sd[b0:b0 + ROWS].T)                    # [K, ROWS]
        ctx_c = np.ascontiguousarray(
            ctx[b0:b0 + ROWS].reshape(NT, NB, P, D).transpose(2, 0, 1, 3)
        ).reshape(P, NT * NB * D)
        in_maps.append({"sd": sd_c, "ctx": ctx_c, "c_all": c_all})

    results = run_bass_kernel_spmd(nc, in_maps, core_ids=list(range(NCORES)),
                                   trace=trace, **kw)
    out = np.concatenate(
        [results.results[c]["out"].astype(np.float32) for c in range(NCORES)],
        axis=0)
    return out, results


def kernel(**inputs) -> np.ndarray:
    out, _ = run(inputs)
    return out
